# revision 20
# baseline (speedup 1.0000x reference)
# Trainium2 Bass kernel for nn_MicroVideoRec (segment_reduce).
#
# Strategy (8 NeuronCores, SPMD):
#   Host: bucket the 20M interactions by item_id into fixed-width per-bin
#     slots.  Each bin (item) is assigned a compile-time width class
#     W in {16,20,24,28,32,40,64} (smallest W >= bin count; pad factor
#     ~1.08x) and a fixed (core, partition, slot-range).  Three element
#     arrays are built per core:
#       s32 [128, E_PP] f32 : signal values; PAD SLOTS HOLD THE BIN MAX
#            (neutral for both grouped max and grouped min, so
#            mpos/mneg are bit-exact f32 == reference, making the
#            |mpos|>=|mneg| tie-break exact)
#       m16 [128, E_PP] fp16: 1.0 for real elements, 0.0 pads
#       r16 [128, E_PP] fp16: rep values, 0.0 pads
#   Device (per core): for each width class, grouped reductions over the
#     fixed W-slot groups: mpos=max(s32), mneg=min(s32), Ss=sum(s32),
#     cnt=sum(m16), Srep=sum(r16)  (fp16 sums use two pairwise folds in
#     DVE 2x mode first).  sig_sum = Ss + (cnt-W)*mpos undoes the pad
#     contribution.  Dense epilogue computes both output channels; a
#     16-float AllReduce shares the rep_log sum/sumsq for global mean/std.
#   Host: gathers the 8 per-core [2, 129280] outputs back to bin order.
import sys
import numpy as np

try:
    import concourse.bass as bass
except ImportError:  # pragma: no cover
    sys.path.insert(0, "/opt/trn_rl_repo")
    import concourse.bass as bass

import concourse.bacc as bacc
import concourse.tile as tile
from concourse import mybir
from concourse.bass_utils import run_bass_kernel_spmd

PREP_TAG = "v5_interleave"

P = 128
NCORES = 8
NROWS = NCORES * P            # 1024 partition rows over all cores
NUM_ITEMS = 1_000_000

# Width classes and per-partition bin capacity per class (compile-time).
WCLS = (16, 20, 24, 28, 32, 64)
NBP = (223, 341, 286, 124, 30, 5)
NSUB = (1, 2, 2, 1, 1, 1)     # bin-chunks per class (SBUF workspace cap)
B_PP = sum(NBP)                       # bins per partition (1010)
E_PP = sum(w * n for w, n in zip(WCLS, NBP))   # elems per partition (21948)
B_CORE = P * B_PP                     # 129280 output bins per core
E_OFF = tuple(int(x) for x in np.cumsum([0] + [w * n for w, n in zip(WCLS, NBP)])[:-1])
B_OFF = tuple(int(x) for x in np.cumsum([0] + list(NBP))[:-1])

f32 = mybir.dt.float32
f16 = mybir.dt.float16
ALU = mybir.AluOpType
ACT = mybir.ActivationFunctionType
AXX = mybir.AxisListType.X


def build_nc(repeat=1, mode="full"):
    nc = bacc.Bacc("TRN2", target_bir_lowering=False, debug=False,
                   num_devices=NCORES)

    s_in = nc.dram_tensor("s_in", [P, E_PP], f32, kind="ExternalInput").ap()
    m_in = nc.dram_tensor("m_in", [P, E_PP], f16, kind="ExternalInput").ap()
    r_in = nc.dram_tensor("r_in", [P, E_PP], f16, kind="ExternalInput").ap()
    lam_in = nc.dram_tensor("lam_in", [P, 1], f32, kind="ExternalInput").ap()

    cc_in = nc.dram_tensor("cc_in", [1, 16], f32).ap()
    cc_out = nc.dram_tensor("cc_out", [1, 16], f32, addr_space="Shared").ap()
    out_d = nc.dram_tensor("out_d", [2, B_CORE], f32,
                           kind="ExternalOutput").ap()

    with tile.TileContext(nc) as tc:
        with tc.tile_pool(name="const", bufs=1) as const_p, \
             tc.tile_pool(name="small", bufs=1) as small_p:
            ones_col = const_p.tile([P, 1], f32)
            nc.vector.memset(ones_col[:], 1.0)
            ones_row = const_p.tile([1, P], f32)
            nc.vector.memset(ones_row[:], 1.0)
            one_bias = const_p.tile([P, 1], f32)
            nc.vector.memset(one_bias[:], 1.0)
            wvec = const_p.tile([P, B_PP], f32)
            for ci, (w, nbp) in enumerate(zip(WCLS, NBP)):
                bo = B_OFF[ci]
                nc.vector.memset(wvec[:, bo:bo + nbp], float(w))

            lamraw_t = small_p.tile([P, 1], f32)
            nc.sync.dma_start(lamraw_t[:], lam_in)
            lam_t = small_p.tile([P, 1], f32)
            nc.scalar.activation(lam_t[:], lamraw_t[:], ACT.Sigmoid)

            for _rep in range(repeat):
                _build_body(nc, tc, s_in, m_in, r_in, cc_in, cc_out, out_d,
                            ones_col, ones_row, one_bias, lam_t, wvec,
                            mode=mode)
    nc.compile()
    return nc


def _emit_chains(nc, work_p, chains, w, nbp, c0, nbc, it):
    """Interleaved slot-major fold chains: reduce w rows of width nbc
    (columns [c0, c0+nbc) of a [P, w*nbp] class region) down to each
    chain's [P, nbc] output with pairwise tensor_tensor ops.  Levels are
    emitted round-robin across chains so every instruction's producer is
    several instructions back (hides semaphore latency).
    """
    assert w % 2 == 0
    states = []
    for (name, tile_ap, op, dt, out_slice, f32_tail) in chains:
        states.append({"name": name, "tile": tile_ap, "op": op, "dt": dt,
                       "out": out_slice, "tail": f32_tail,
                       "rows": w, "src": None, "lvl": 0})
    while True:
        active = [s for s in states if s["rows"] > 1]
        if not active:
            break
        absorbs = []
        for s in active:
            rows = s["rows"]
            half = rows // 2
            odd = rows % 2
            last = half == 1
            use_f32 = last or (half <= s["tail"])
            dtype = f32 if (use_f32 and s["dt"] == f16) else s["dt"]
            if last:
                dst = s["out"]
            else:
                dst = work_p.tile(
                    [P, half * nbc], dtype,
                    tag=f"w{s['name']}{s['lvl'] % 2}",
                    name=f"ch_{s['name']}_{it}_{s['lvl']}")[:]
            if s["src"] is None:
                reg = s["tile"][:, 0:w * nbp].rearrange(
                    "p (w b) -> p w b", b=nbp)
                dv = dst.rearrange("p (h b) -> p h b", b=nbc)
                nc.vector.tensor_tensor(
                    out=dv, in0=reg[:, 0:half, c0:c0 + nbc],
                    in1=reg[:, half:2 * half, c0:c0 + nbc], op=s["op"])
            else:
                src_ap = s["src"]
                nc.vector.tensor_tensor(
                    out=dst, in0=src_ap[:, 0:half * nbc],
                    in1=src_ap[:, half * nbc:2 * half * nbc], op=s["op"])
                if odd:
                    absorbs.append(
                        (dst, src_ap[:, 2 * half * nbc:rows * nbc], s["op"]))
            s["src"] = dst
            s["rows"] = half
            s["lvl"] += 1
        for dst, extra, op in absorbs:
            nc.vector.tensor_tensor(out=dst[:, 0:nbc], in0=dst[:, 0:nbc],
                                    in1=extra, op=op)


def _build_body(nc, tc, s_in, m_in, r_in, cc_in, cc_out, out_d,
                ones_col, ones_row, one_bias, lam_t, wvec, mode="full"):
    with tc.tile_pool(name="acc", bufs=1) as acc_p:
        Cnt = acc_p.tile([P, B_PP], f32, name="Cnt")
        Ssig = acc_p.tile([P, B_PP], f32, name="Ssig")
        Srep = acc_p.tile([P, B_PP], f32, name="Srep")
        Mpos = acc_p.tile([P, B_PP], f32, name="Mpos")
        Mneg = acc_p.tile([P, B_PP], f32, name="Mneg")

        with tc.tile_pool(name="in", bufs=2) as in_p, \
             tc.tile_pool(name="work", bufs=1) as work_p:
            for ci, (w, nbp) in enumerate(zip(WCLS, NBP)):
                sz = w * nbp
                eo, bo = E_OFF[ci], B_OFF[ci]
                st = in_p.tile([P, sz], f32, tag="s", name=f"s{ci}")
                nc.sync.dma_start(st[:], s_in[:, eo:eo + sz])
                mt = in_p.tile([P, sz], f16, tag="m", name=f"m{ci}")
                nc.sync.dma_start(mt[:], m_in[:, eo:eo + sz])
                rt = in_p.tile([P, sz], f16, tag="r", name=f"r{ci}")
                nc.sync.dma_start(rt[:], r_in[:, eo:eo + sz])

                if mode == "dma":
                    continue
                if mode == "tt1":
                    tt_t = work_p.tile([P, sz], f32, tag="wx0",
                                       name=f"tt{ci}")
                    nc.vector.tensor_tensor(out=tt_t[:], in0=st[:],
                                            in1=st[:], op=ALU.add)
                    continue
                if mode == "tt1h":
                    th_t = work_p.tile([P, sz], f16, tag="wc0",
                                       name=f"tth{ci}")
                    nc.vector.tensor_tensor(out=th_t[:], in0=mt[:],
                                            in1=mt[:], op=ALU.add)
                    continue
                if mode == "red1":
                    rd_t = work_p.tile([P, 1], f32, tag="rd",
                                       name=f"rd{ci}")
                    nc.vector.tensor_reduce(out=rd_t[:], in_=st[:],
                                            axis=AXX, op=ALU.add)
                    continue
                nsub = NSUB[ci]
                csz = (nbp + nsub - 1) // nsub
                for si in range(nsub):
                    c0 = si * csz
                    nbc = min(csz, nbp - c0)
                    ob = slice(bo + c0, bo + c0 + nbc)
                    _emit_chains(
                        nc, work_p,
                        [("x", st[:], ALU.max, f32, Mpos[:, ob], 0),
                         ("n", st[:], ALU.min, f32, Mneg[:, ob], 0),
                         ("s", st[:], ALU.add, f32, Ssig[:, ob], 0),
                         ("c", mt[:], ALU.add, f16, Cnt[:, ob], 0),
                         ("r", rt[:], ALU.add, f16, Srep[:, ob], 2)],
                        w, nbp, c0, nbc, f"{ci}_{si}")

        if mode in ("dma", "tt1", "tt1h", "red1"):
            nc.vector.memset(Ssig[:], 0.0)
            nc.vector.memset(Srep[:], 0.0)
        if mode in ("dma", "stream", "tt1", "tt1h", "red1"):
            nc.sync.dma_start(out_d[0].rearrange("(p j) -> p j", p=P),
                              Ssig[:])
            nc.sync.dma_start(out_d[1].rearrange("(p j) -> p j", p=P),
                              Srep[:])
            return

        # ---- epilogue ----
        # rep-stats path first so the AllReduce overlaps the signal path.
        with tc.tile_pool(name="epi", bufs=1) as epi_p, \
             tc.tile_pool(name="psum", bufs=1, space="PSUM") as psum_p:
            B = B_PP
            safe_t = epi_p.tile([P, B], f32)
            nc.vector.tensor_scalar(out=safe_t[:], in0=Cnt[:], scalar1=1.0,
                                    scalar2=None, op0=ALU.max)
            inv_t = epi_p.tile([P, B], f32)
            nc.vector.reciprocal(inv_t[:], safe_t[:])
            repmean_t = epi_p.tile([P, B], f32)
            nc.vector.tensor_tensor(out=repmean_t[:], in0=Srep[:],
                                    in1=inv_t[:], op=ALU.mult)
            replog_t = epi_p.tile([P, B], f32)
            s1_t = epi_p.tile([P, 1], f32)
            nc.scalar.activation(replog_t[:], repmean_t[:], ACT.Ln,
                                 bias=one_bias[:], accum_out=s1_t[:])
            sq_t = epi_p.tile([P, B], f32)
            s2_t = epi_p.tile([P, 1], f32)
            nc.scalar.activation(sq_t[:], replog_t[:], ACT.Square,
                                 accum_out=s2_t[:])
            s12_t = epi_p.tile([P, 16], f32)
            nc.vector.memset(s12_t[:], 0.0)
            nc.vector.tensor_copy(out=s12_t[:, 0:1], in_=s1_t[:])
            nc.vector.tensor_copy(out=s12_t[:, 1:2], in_=s2_t[:])
            red_ps = psum_p.tile([1, 16], f32, space="PSUM")
            nc.tensor.matmul(out=red_ps[:], lhsT=ones_col[:], rhs=s12_t[:],
                             start=True, stop=True)
            red_sb = epi_p.tile([1, 16], f32)
            nc.vector.tensor_copy(out=red_sb[:], in_=red_ps[:])
            nc.sync.dma_start(cc_in, red_sb[:])
            if mode != "nocc":
                nc.gpsimd.collective_compute(
                    "AllReduce", ALU.add,
                    replica_groups=[list(range(NCORES))],
                    ins=[cc_in], outs=[cc_out])

            # signal channel (overlaps the collective)
            # sig_sum = Ss + (cnt - W) * mpos  (undo pad contribution)
            tw_t = epi_p.tile([P, B], f32)
            nc.vector.tensor_tensor(out=tw_t[:], in0=Cnt[:], in1=wvec[:],
                                    op=ALU.subtract)
            nc.vector.tensor_tensor(out=tw_t[:], in0=tw_t[:], in1=Mpos[:],
                                    op=ALU.mult)
            nc.vector.tensor_tensor(out=tw_t[:], in0=tw_t[:], in1=Ssig[:],
                                    op=ALU.add)
            sigmean_t = epi_p.tile([P, B], f32)
            nc.vector.tensor_tensor(out=sigmean_t[:], in0=tw_t[:],
                                    in1=inv_t[:], op=ALU.mult)
            absP_t = epi_p.tile([P, B], f32)
            nc.scalar.activation(absP_t[:], Mpos[:], ACT.Abs)
            absN_t = epi_p.tile([P, B], f32)
            nc.scalar.activation(absN_t[:], Mneg[:], ACT.Abs)
            ge_t = epi_p.tile([P, B], mybir.dt.int32)
            nc.vector.tensor_tensor(out=ge_t[:], in0=absP_t[:],
                                    in1=absN_t[:], op=ALU.is_ge)
            maxabs_t = epi_p.tile([P, B], f32)
            nc.vector.tensor_copy(out=maxabs_t[:], in_=Mneg[:])
            nc.vector.copy_predicated(out=maxabs_t[:], mask=ge_t[:],
                                      data=Mpos[:])
            sigfull_t = epi_p.tile([P, B], f32)
            nc.vector.scalar_tensor_tensor(
                out=sigfull_t[:], in0=maxabs_t[:], scalar=lam_t[:],
                in1=sigmean_t[:], op0=ALU.mult, op1=ALU.add)
            nc.sync.dma_start(out_d[0].rearrange("(p j) -> p j", p=P),
                              sigfull_t[:])

            # collective result -> global mean/std -> rep channel
            tot_sb = epi_p.tile([1, 16], f32)
            nc.sync.dma_start(tot_sb[:], cc_out if mode != "nocc" else cc_in)
            tot_ps = psum_p.tile([P, 16], f32, space="PSUM")
            nc.tensor.matmul(out=tot_ps[:], lhsT=ones_row[:], rhs=tot_sb[:],
                             start=True, stop=True)
            tot_t = epi_p.tile([P, 16], f32)
            nc.vector.tensor_copy(out=tot_t[:], in_=tot_ps[:])

            NB = float(NUM_ITEMS)
            mean_t = epi_p.tile([P, 1], f32)
            nc.vector.tensor_scalar(out=mean_t[:], in0=tot_t[:, 0:1],
                                    scalar1=1.0 / NB, scalar2=None,
                                    op0=ALU.mult)
            m2s_t = epi_p.tile([P, 1], f32)
            nc.vector.tensor_tensor(out=m2s_t[:], in0=mean_t[:],
                                    in1=tot_t[:, 0:1], op=ALU.mult)
            var_t = epi_p.tile([P, 1], f32)
            nc.vector.tensor_tensor(out=var_t[:], in0=tot_t[:, 1:2],
                                    in1=m2s_t[:], op=ALU.subtract)
            nc.vector.tensor_scalar(out=var_t[:], in0=var_t[:],
                                    scalar1=1.0 / (NB - 1.0), scalar2=None,
                                    op0=ALU.mult)
            # std = sqrt(var) = exp(0.5*ln(var)): stays in the same
            # activation-function table set as Ln/Abs/Square (no reload)
            lnv_t = epi_p.tile([P, 1], f32)
            nc.scalar.activation(lnv_t[:], var_t[:], ACT.Ln)
            std_t = epi_p.tile([P, 1], f32)
            nc.scalar.activation(std_t[:], lnv_t[:], ACT.Exp, scale=0.5)
            nc.vector.tensor_scalar(out=std_t[:], in0=std_t[:], scalar1=1e-6,
                                    scalar2=None, op0=ALU.add)
            istd_t = epi_p.tile([P, 1], f32)
            nc.vector.reciprocal(istd_t[:], std_t[:])
            repsc_t = epi_p.tile([P, B], f32)
            nc.vector.tensor_scalar(out=repsc_t[:], in0=replog_t[:],
                                    scalar1=mean_t[:], scalar2=istd_t[:],
                                    op0=ALU.subtract, op1=ALU.mult)
            nc.sync.dma_start(out_d[1].rearrange("(p j) -> p j", p=P),
                              repsc_t[:])


def host_prep(item_ids, signals, reps):
    """Bucket elements into fixed-width per-bin slots; build s/m/r arrays.

    Returns (s32, m16, r16) each [NCORES, P, E_PP], and gpos [NUM_ITEMS]
    mapping bin -> column in the concatenated [2, NCORES*B_CORE] output.
    """
    ids = np.asarray(item_ids).astype(np.int32)
    sig = np.asarray(signals, dtype=np.float32)
    rep = np.asarray(reps, dtype=np.float32)

    cnt = np.bincount(ids, minlength=NUM_ITEMS).astype(np.int32)
    assert cnt.max() <= WCLS[-1], f"bin count {cnt.max()} > {WCLS[-1]}"
    Wa = np.asarray(WCLS, np.int32)
    cls = np.searchsorted(Wa, cnt, side="left").astype(np.int32)

    row_of = np.empty(NUM_ITEMS, np.int32)
    j_of = np.empty(NUM_ITEMS, np.int32)
    for c in range(len(WCLS)):
        binsc = np.flatnonzero(cls == c)
        capc = NBP[c] * NROWS
        if len(binsc) > capc:
            assert c + 1 < len(WCLS), "largest width class overflowed"
            cls[binsc[capc:]] = c + 1
            binsc = binsc[:capc]
        k = np.arange(len(binsc), dtype=np.int32)
        row_of[binsc] = k % NROWS
        j_of[binsc] = k // NROWS

    e_off = np.asarray(E_OFF, np.int32)
    b_off = np.asarray(B_OFF, np.int32)
    wbin = Wa[cls]                              # slot width per bin
    nbp_of = np.asarray(NBP, np.int32)[cls]     # slot stride (slot-major)
    # slot-major within class region: addr = base + slot_idx * nbp + j
    base = row_of * np.int32(E_PP) + e_off[cls] + j_of

    order = np.argsort(ids)
    ids_s = ids[order]
    starts = np.zeros(NUM_ITEMS + 1, np.int64)
    np.cumsum(cnt, out=starts[1:])
    ranks = (np.arange(len(ids), dtype=np.int64) - starts[ids_s]).astype(
        np.int32)
    flat = base[ids_s] + ranks * nbp_of[ids_s]

    sig_s = sig[order]
    rep_s = rep[order]
    # per-bin max (pads of s32); empty bins -> 0
    mpos = np.zeros(NUM_ITEMS, np.float32)
    ne = cnt > 0
    mpos[ne] = np.maximum.reduceat(sig_s, starts[:-1][ne])

    # fill only the pad slots [cnt, w) of each bin with the bin max
    npad = wbin - cnt                            # pads per bin (>=0)
    pad_bin = np.repeat(np.arange(NUM_ITEMS, dtype=np.int32), npad)
    pstarts = np.zeros(NUM_ITEMS + 1, np.int64)
    np.cumsum(npad, out=pstarts[1:])
    pwithin = (np.arange(int(pstarts[-1]), dtype=np.int64)
               - pstarts[pad_bin]).astype(np.int32)
    pad_flat = (base[pad_bin]
                + (cnt[pad_bin] + pwithin) * nbp_of[pad_bin])

    s = np.zeros(NROWS * E_PP, np.float32)
    s[pad_flat] = mpos[pad_bin]
    s[flat] = sig_s
    m = np.zeros(NROWS * E_PP, np.float16)
    m[flat] = 1.0
    r = np.zeros(NROWS * E_PP, np.float16)
    r[flat] = rep_s.astype(np.float16)
    shape = (NCORES, P, E_PP)
    gpos = row_of.astype(np.int64) * B_PP + b_off[cls] + j_of
    return s.reshape(shape), m.reshape(shape), r.reshape(shape), gpos


_NC_CACHE = {}
_GPOS = {"gpos": None}


def _get_nc(repeat=1):
    if repeat not in _NC_CACHE:
        _NC_CACHE[repeat] = build_nc(repeat)
    return _NC_CACHE[repeat]


def make_in_maps(item_ids, signals, reps, lam_raw):
    s, m, r, gpos = host_prep(item_ids, signals, reps)
    _GPOS["gpos"] = gpos
    lam_vec = np.full((P, 1), float(np.asarray(lam_raw)), np.float32)
    in_maps = []
    for k in range(NCORES):
        in_maps.append({
            "s_in": np.ascontiguousarray(s[k]),
            "m_in": np.ascontiguousarray(m[k]),
            "r_in": np.ascontiguousarray(r[k]),
            "lam_in": lam_vec,
        })
    return in_maps


def run_maps(in_maps, repeat=1, trace=False):
    nc = _get_nc(repeat)
    res = run_bass_kernel_spmd(nc, in_maps, core_ids=list(range(NCORES)),
                               trace=trace)
    outs = [res.results[k]["out_d"] for k in range(NCORES)]
    cat = np.concatenate(outs, axis=1)
    full = cat[:, _GPOS["gpos"]].astype(np.float32)
    if trace:
        return full, res
    return full


def kernel(item_ids, signals, reps, lam_raw, num_items=None, _repeat=1):
    if num_items is not None:
        assert int(num_items) == NUM_ITEMS
    return run_maps(make_in_maps(item_ids, signals, reps, lam_raw), _repeat)
